# revision 44
# baseline (speedup 1.0000x reference)
"""Trainium2 Bass kernel for nn_ALALLaDA windowed-MoE routing blend.

Reference math (see reference.py): out = h + ALPHA * delta, where delta is
written only at masked positions a with >=1 unmasked neighbor in the +-r
window, and equals layer_norm_d of
    mean_t ( sum_k w[a,k] * MLP_k(h[t]) )  over unmasked neighbors t != a,
with w = softmax(h Wr + br) taken at the DESTINATION position a.

Distribution: data-parallel over tokens across 8 cores. The global list of
output positions (masked, >=1 valid neighbor) is split into 8 balanced
contiguous groups (may straddle the batch-row boundary); each core gets the
group plus the +-r source halo -> no collectives.

Algorithmic structure (per core, all layout/indexing prep on host):
  - sources: only unmasked tokens matter (validity 0 otherwise),
    host-compacted,
  - outputs: host-compacted masked positions (~256/core -> full 128-tiles),
  - mm1 (token-major): act0 = gelu(h W1 + b1) for all experts at source
    tokens; b1 rides as an extra contraction row of (h|1)(W1|b1),
  - window sum + destination routing weights fold into one host-built
    banded matrix per expert, At_k[t,a] = band[t,a] * w[a,k], applied by
    associativity BEFORE the W2 contraction:
        Y[(k,f),a] = sum_t act0[t,(k,f)] At_k[t,a]      (cheap matmul)
        num[a,:]   = sum_kf Y[(k,f),a] W2[(k,f),:]      (main matmul 2)
    so no [B,S,K,D] tensor and no device-side routing math at all,
  - 1/cnt normalization cancels inside layer_norm (scale invariance, eps
    perturbation ~1e-3 relative on delta), delta = (num-mu)*ALPHA/std,
  - device returns compacted delta rows (bf16); host scatter-adds into h.
Compute dtype bf16 (weights/activations), f32 PSUM accumulation.
"""

import sys

import numpy as np
import ml_dtypes

if "/opt/trn_rl_repo" not in sys.path:
    sys.path.insert(0, "/opt/trn_rl_repo")

ALPHA = 0.08
EPS = 1e-5
N_CORES = 8
P = 128

BF16 = ml_dtypes.bfloat16
FP8 = ml_dtypes.float8_e4m3fn
USE_FP8 = True     # fp8 DoubleRow for the two big matmuls
W1_SCALE = 16.0    # keep fp8 weights out of subnormal range
W2_SCALE = 16.0
Y_SCALE = 8.0      # ysb scale (folded: = AM_SCALE, evict is a plain copy)
AM_SCALE = 8.0     # banded-routing matrix scale for fp8 range


def _ceil_div(a, b):
    return (a + b - 1) // b


def _rup(x, m):
    return _ceil_div(x, m) * m


def _host_prep(h_L, mask, Wr, br, W1, b1, W2, b2, r):
    """Shard + compact on host. Returns (in_maps, dims, out indices)."""
    h_L = np.asarray(h_L, np.float32)
    mask = np.asarray(mask)
    B, S, D = h_L.shape
    K, _, Fh = np.asarray(W1).shape
    h_flat = h_L.reshape(B * S, D)

    masked = mask != 0
    um = ~masked
    umf = um.astype(np.int64)

    # neighbor count per position (excluding center), same clipping as ref
    cnt = np.zeros((B, S), np.int64)
    for o in range(-r, r + 1):
        if o == 0:
            continue
        if o > 0:
            cnt[:, : S - o] += umf[:, o:]
        else:
            cnt[:, -o:] += umf[:, :o]

    b1_nz = bool(np.any(np.asarray(b1)))
    b2_nz = bool(np.any(np.asarray(b2)))

    # balanced split of the global output list into 8 contiguous groups
    glob_out = np.nonzero((masked & (cnt > 0)).reshape(-1))[0]
    n_out = len(glob_out)
    base, rem = divmod(n_out, N_CORES)
    sizes = [base + (1 if i < rem else 0) for i in range(N_CORES)]
    bounds = np.concatenate([[0], np.cumsum(sizes)])

    out_idx, src_idx = [], []
    for c in range(N_CORES):
        oidx = glob_out[bounds[c]: bounds[c + 1]]
        out_idx.append(oidx)
        srcs = []
        for b in np.unique(oidx // S) if len(oidx) else []:
            seg = oidx[oidx // S == b] % S
            lo, hi = max(int(seg.min()) - r, 0), min(int(seg.max()) + r + 1, S)
            srcs.append(np.nonzero(um[b, lo:hi])[0] + lo + b * S)
        src_idx.append(np.concatenate(srcs) if srcs
                       else np.zeros(0, np.int64))

    T_pad = max(32, _rup(max(len(i) for i in src_idx), 32))
    A_pad = max(32, _rup(max(len(i) for i in out_idx), 32))

    D_ext = D + (1 if b1_nz else 0)
    wdt = FP8 if USE_FP8 else BF16
    wsc1 = W1_SCALE if USE_FP8 else 1.0
    wsc2 = W2_SCALE if USE_FP8 else 1.0
    w1r = np.transpose(np.asarray(W1, np.float32), (1, 0, 2)).reshape(D, K * Fh)
    if b1_nz:
        w1r = np.concatenate(
            [w1r, np.asarray(b1, np.float32).reshape(1, K * Fh)], 0)
    w1r = np.ascontiguousarray(w1r * wsc1).astype(wdt)
    w2r = np.asarray(W2, np.float32).reshape(K * Fh, D)
    if b2_nz:
        w2r = np.concatenate([w2r, np.asarray(b2, np.float32).reshape(K, D)], 0)
    w2r = np.ascontiguousarray(w2r * wsc2).astype(wdt)
    Wrf = np.asarray(Wr, np.float32)
    brf = np.asarray(br, np.float32)
    cnt_flat = cnt.reshape(-1)

    in_maps = []
    for c in range(N_CORES):
        sidx, oidx = src_idx[c], out_idx[c]
        Tu, Au = len(sidx), len(oidx)
        hT = np.zeros((D_ext, T_pad), np.float32)
        hT[:D, :Tu] = h_flat[sidx, :].T
        if b1_nz:
            hT[D, :Tu] = 1.0
        # destination routing softmax on host (0.1% of the flops, exact f32)
        logits = h_flat[oidx, :] @ Wrf.T + brf
        logits -= logits.max(1, keepdims=True)
        ew = np.exp(logits)
        w = ew / ew.sum(1, keepdims=True)            # [Au, K]
        # banded per-expert matrices At_k[t, a] = band * w[a, k];
        # same-row check: cross-row global diffs only pass |diff|<=r at the
        # row seam, which the row-equality term rejects.
        band = (np.abs(sidx[:, None] - oidx[None, :]) <= r) & \
               (sidx[:, None] != oidx[None, :]) & \
               ((sidx[:, None] // S) == (oidx[None, :] // S))
        am = np.zeros((K, T_pad, A_pad), np.float32)
        am[:, :Tu, :Au] = band[None, :, :] * w.T[:, None, :]
        entry = {
            "hT": np.ascontiguousarray(hT.astype(wdt)),
            "amat": np.ascontiguousarray(
                (am * AM_SCALE).astype(wdt) if USE_FP8 else am.astype(BF16)),
            "w1": w1r,
            "w2": w2r,
        }
        if b2_nz:
            wc = np.zeros((K, A_pad), np.float32)
            wc[:, :Au] = (w * cnt_flat[oidx][:, None]).T
            if USE_FP8:
                wc *= Y_SCALE
            entry["wcnt"] = np.ascontiguousarray(wc.astype(wdt))
        in_maps.append(entry)

    dims = dict(B=B, S=S, D=D, K=K, Fh=Fh, T_pad=T_pad, A_pad=A_pad,
                D_ext=D_ext, b1_nz=b1_nz, b2_nz=b2_nz)
    return in_maps, dims, out_idx


def _build(dims):
    import concourse.tile as tile
    from concourse import bacc, mybir
    from contextlib import ExitStack

    D, K, Fh = dims["D"], dims["K"], dims["Fh"]
    T_pad, A_pad, D_ext = dims["T_pad"], dims["A_pad"], dims["D_ext"]
    b2_nz = dims["b2_nz"]
    KF = K * Fh
    NKF = KF // P            # kf-chunks (32)
    ND = _ceil_div(D_ext, P)   # contraction chunks of mm1
    TCH = _ceil_div(T_pad, P)  # source-token chunks
    ACH = _ceil_div(A_pad, P)  # output-token chunks
    NQ = D // 512            # 512-wide column tiles of num
    KF2 = NKF + (1 if b2_nz else 0)
    AS = _ceil_div(A_pad, 512)  # N-slices for Y (A_pad normally <= 512)

    def tsz(t):
        return min(P, T_pad - t * P)

    def asz(m):
        return min(P, A_pad - m * P)

    def dsz(d):
        return min(P, D_ext - d * P)

    DT16 = mybir.dt.bfloat16
    DTF = mybir.dt.float32
    DTW = mybir.dt.float8e4 if USE_FP8 else DT16
    DR = mybir.MatmulPerfMode.DoubleRow if USE_FP8 else None
    F = mybir.ActivationFunctionType

    nc = bacc.Bacc()
    hT_ext = nc.declare_dram_parameter("hT", [D_ext, T_pad], DTW, isOutput=False)
    am_ext = nc.declare_dram_parameter("amat", [K, T_pad, A_pad], DTW,
                                       isOutput=False)
    w1_ext = nc.declare_dram_parameter("w1", [D_ext, KF], DTW, isOutput=False)
    w2_ext = nc.declare_dram_parameter("w2", [KF + (K if b2_nz else 0), D],
                                       DTW, isOutput=False)
    if b2_nz:
        wc_ext = nc.declare_dram_parameter("wcnt", [K, A_pad], DTW,
                                           isOutput=False)
    out_ext = nc.declare_dram_parameter("out", [A_pad, D], DT16, isOutput=True)

    with tile.TileContext(nc) as tc, ExitStack() as ctx:
        const = ctx.enter_context(tc.tile_pool(name="const", bufs=1))

        hsb = const.tile([P, ND, T_pad], DTW)
        for d in range(ND):
            # issue from the (otherwise idle) gpsimd queue so the sync
            # queue can trigger the first W1 slabs without waiting behind
            # 16 serialized descriptor preps (~0.7us each)
            nc.gpsimd.dma_start(hsb[: dsz(d), d, :],
                                hT_ext[d * P: d * P + dsz(d), :])
        amsb = const.tile([P, TCH, K, A_pad], DTW)
        epssb = const.tile([P, 1], DTF)
        if b2_nz:
            wcsb = const.tile([K, A_pad], DTW)

        act0 = const.tile([P, TCH, KF], DTW)     # gelu acts, token-major
        ysb = const.tile([P, NKF, A_pad], DTW)   # banded-mixed activations

        # contraction steps for mm1: DoubleRow pairs of full 128-chunks,
        # singles for any leftover (e.g. the b1 extra row)
        dsteps = []
        d = 0
        while d < ND:
            if USE_FP8 and d + 1 < ND and dsz(d) == P and dsz(d + 1) == P:
                dsteps.append((d, 2))
                d += 2
            else:
                dsteps.append((d, 1))
                d += 1

        # ---- phase 1+2: mm1 (h W1 -> gelu) and Y (act0 @ At) interleaved ---
        KK = 1024  # kf columns per W1 streaming pass
        NKK = KF // KK
        with tc.tile_pool(name="ps_1", bufs=6, space="PSUM") as ps_1, \
             tc.tile_pool(name="ps_y", bufs=2, space="PSUM") as ps_y, \
             tc.tile_pool(name="w1p", bufs=2 * len(dsteps) + 2) as w1p:
            for kk in range(NKK):
                slabs = {}
                for si, (d0, nsub) in enumerate(dsteps):
                    rows = dsz(d0) if nsub == 1 else 2 * P
                    s = w1p.tile([P, nsub, KK], DTW, tag="w1s",
                                 name=f"w1s_{kk}_{si}")
                    src = w1_ext[d0 * P: d0 * P + rows,
                                 kk * KK:(kk + 1) * KK]
                    if nsub == 2:
                        src = src.rearrange("(two p) n -> p two n", two=2)
                        if kk == 0 and si == 0:
                            # first slab gates the first matmul: two queues
                            nc.sync.dma_start(s[:, 0:1, :],
                                              src[:, 0:1, :])
                            nc.sync.dma_start(s[:, 1:2, :],
                                              src[:, 1:2, :])
                        else:
                            nc.sync.dma_start(s[:], src)
                    else:
                        nc.sync.dma_start(s[: dsz(d0), 0, :], src)
                    slabs[si] = s
                if kk == 0:
                    # side inputs ride the scalar/vector trigger queues so
                    # the sync queue keeps streaming W1
                    for t in range(TCH):
                        for k in range(K):
                            eng = nc.scalar if (t * K + k) % 2 else nc.gpsimd
                            eng.dma_start(
                                amsb[: tsz(t), t, k, :],
                                am_ext[k, t * P: t * P + tsz(t), :])
                    if b2_nz:
                        nc.scalar.dma_start(wcsb[:], wc_ext[:])
                    nc.vector.memset(epssb[:], EPS)
                for m in range(TCH):
                    mp = tsz(m)
                    for q in range(KK // 512):
                        pt = ps_1.tile([P, 512], DTF, tag="pt",
                                       name=f"pt_{kk}_{m}_{q}")
                        for si, (d0, nsub) in enumerate(dsteps):
                            if nsub == 2:
                                nc.tensor.matmul(
                                    pt[:mp, :],
                                    hsb[:, d0: d0 + 2, m * P: m * P + mp],
                                    slabs[si][:, :, q * 512:(q + 1) * 512],
                                    start=(si == 0),
                                    stop=(si == len(dsteps) - 1),
                                    perf_mode=DR)
                            else:
                                nc.tensor.matmul(
                                    pt[:mp, :],
                                    hsb[: dsz(d0), d0, m * P: m * P + mp],
                                    slabs[si][: dsz(d0), 0,
                                              q * 512:(q + 1) * 512],
                                    start=(si == 0),
                                    stop=(si == len(dsteps) - 1))
                        nc.scalar.activation(
                            act0[:mp, m, kk * KK + q * 512:
                                 kk * KK + (q + 1) * 512],
                            pt[:mp, :], F.Gelu,
                            scale=(1.0 / W1_SCALE) if USE_FP8 else 1.0)
                # Y for the kf-chunks covered by this pass
                tsteps = []
                t = 0
                while t < TCH:
                    if USE_FP8 and t + 1 < TCH and tsz(t) == P \
                            and tsz(t + 1) == P:
                        tsteps.append((t, 2))
                        t += 2
                    else:
                        tsteps.append((t, 1))
                        t += 1
                for cc in range(KK // P):
                    cidx = kk * (KK // P) + cc
                    k = cidx // (Fh // P)
                    for ns in range(AS):
                        n0, n1 = ns * 512, min((ns + 1) * 512, A_pad)
                        py = ps_y.tile([P, min(512, A_pad)], DTF, tag="py",
                                       name=f"py_{cidx}_{ns}")
                        for ti, (t0, nsub) in enumerate(tsteps):
                            if nsub == 2:
                                nc.tensor.matmul(
                                    py[:, : n1 - n0],
                                    act0[:, t0: t0 + 2,
                                         cidx * P:(cidx + 1) * P],
                                    amsb[:, t0: t0 + 2, k, n0:n1],
                                    start=(ti == 0),
                                    stop=(ti == len(tsteps) - 1),
                                    perf_mode=DR)
                            else:
                                pp = tsz(t0)
                                nc.tensor.matmul(
                                    py[:, : n1 - n0],
                                    act0[:pp, t0, cidx * P:(cidx + 1) * P],
                                    amsb[:pp, t0, k, n0:n1],
                                    start=(ti == 0),
                                    stop=(ti == len(tsteps) - 1))
                        nc.vector.tensor_copy(ysb[:, cidx, n0:n1],
                                              py[:, : n1 - n0])

        # ---- phase 3: num = Y^T W2 (+ wcnt b2), layernorm, delta out -------
        # contraction steps over kf-chunks (+ optional b2 row-chunk)
        csteps = []
        c = 0
        while c < KF2:
            if USE_FP8 and c + 1 < NKF:
                csteps.append((c, 2))
                c += 2
            else:
                csteps.append((c, 1))
                c += 1
        MG = 3   # output m-tiles per W2 streaming pass (psum: 2*MG+2 <= 8)
        CG = 8   # steps kept resident -> 8 consecutive same-psum matmuls
        sgroups = [list(range(g, min(g + CG, len(csteps))))
                   for g in range(0, len(csteps), CG)]
        with tc.tile_pool(name="ps_2", bufs=8, space="PSUM") as ps_2, \
             tc.tile_pool(name="w2p", bufs=2 * CG + 2) as w2p, \
             tc.tile_pool(name="epi", bufs=2) as epi, \
             tc.tile_pool(name="small", bufs=3) as small:
            for mg0 in range(0, ACH, MG):
                mts = list(range(mg0, min(mg0 + MG, ACH)))
                nums = {m: epi.tile([P, D], DTF, tag="num", name=f"num_{m}")
                        for m in mts}
                stats_t = {m: small.tile([P, NQ, 6], DTF, tag="stats",
                                         name=f"stats_{m}") for m in mts}
                for nh in range(NQ // 2):
                    pst = {}
                    for m in mts:
                        for q in range(2):
                            pst[(m, q)] = ps_2.tile([P, 512], DTF, tag="ps2",
                                                    name=f"ps2_{m}_{q}")
                    for sg in sgroups:
                        slabs = {}
                        for si in sg:
                            c0, nsub = csteps[si]
                            pp = P if c0 < NKF else K
                            s = w2p.tile([P, nsub, 1024], DTW, tag="w2s",
                                         name=f"w2s_{nh}_{si}")
                            src = w2_ext[c0 * P: c0 * P + nsub * pp,
                                         nh * 1024:(nh + 1) * 1024]
                            if nsub == 2:
                                src = src.rearrange("(two p) n -> p two n",
                                                    two=2)
                                nc.sync.dma_start(s[:], src)
                            else:
                                nc.sync.dma_start(s[:pp, 0, :], src)
                            slabs[si] = s
                        for m in mts:
                            mp = asz(m)
                            for q in range(2):
                                for si in sg:
                                    c0, nsub = csteps[si]
                                    start = (si == 0)
                                    stop = (si == len(csteps) - 1)
                                    if nsub == 2:
                                        nc.tensor.matmul(
                                            pst[(m, q)][:mp, :],
                                            ysb[:, c0: c0 + 2,
                                                m * P: m * P + mp],
                                            slabs[si][:, :,
                                                      q * 512:(q + 1) * 512],
                                            start=start, stop=stop,
                                            perf_mode=DR)
                                    else:
                                        pp = P if c0 < NKF else K
                                        if c0 < NKF:
                                            lhs = ysb[:, c0,
                                                      m * P: m * P + mp]
                                        else:
                                            lhs = wcsb[:, m * P: m * P + mp]
                                        nc.tensor.matmul(
                                            pst[(m, q)][:mp, :], lhs,
                                            slabs[si][:pp, 0,
                                                      q * 512:(q + 1) * 512],
                                            start=start, stop=stop)
                    for m in mts:
                        mp = asz(m)
                        for q in range(2):
                            col = nh * 1024 + q * 512
                            nc.scalar.copy(nums[m][:mp, col: col + 512],
                                           pst[(m, q)][:mp, :])
                            # LN stats as soon as each 512-slice is evicted
                            nc.vector.bn_stats(
                                stats_t[m][:mp, 2 * nh + q, :],
                                nums[m][:mp, col: col + 512])
                for m in mts:
                    mp = asz(m)
                    num = nums[m]
                    mv = small.tile([P, 2], DTF, tag="mv", name=f"mv_{m}")
                    nc.vector.bn_aggr(mv[:mp], stats_t[m][:mp])
                    std = small.tile([P, 1], DTF, tag="std", name=f"std_{m}")
                    nc.scalar.activation(std[:mp], mv[:mp, 1:2], F.Sqrt,
                                         bias=epssb[:mp])
                    s2 = small.tile([P, 1], DTF, tag="s2", name=f"s2_{m}")
                    nc.vector.reciprocal(s2[:mp], std[:mp])
                    nc.vector.tensor_scalar_mul(s2[:mp], s2[:mp], ALPHA)
                    scr = epi.tile([P, D], DT16, tag="scr", name=f"scr_{m}")
                    nc.vector.tensor_scalar(scr[:mp], num[:mp], mv[:mp, 0:1],
                                            s2[:mp],
                                            op0=mybir.AluOpType.subtract,
                                            op1=mybir.AluOpType.mult)
                    h0 = mp // 2
                    nc.sync.dma_start(out_ext[m * P: m * P + h0, :],
                                      scr[:h0])
                    if mp > h0:
                        nc.gpsimd.dma_start(
                            out_ext[m * P + h0: m * P + mp, :], scr[h0:mp])

    nc.finalize()
    return nc


def run(inputs, trace=False):
    """Build + execute; returns (full_output, BassKernelResults)."""
    from concourse.bass_utils import run_bass_kernel_spmd

    h_L = np.asarray(inputs["h_L"], np.float32)
    in_maps, dims, out_idx = _host_prep(
        h_L, inputs["mask"], inputs["Wr"], inputs["br"],
        inputs["W1"], inputs["b1"], inputs["W2"], inputs["b2"],
        int(inputs["range_r"]))
    nc = _build(dims)
    res = run_bass_kernel_spmd(nc, in_maps, list(range(N_CORES)), trace=trace)
    out = h_L.copy().reshape(-1, dims["D"])
    for c in range(N_CORES):
        oidx = out_idx[c]
        if len(oidx):
            out[oidx, :] += res.results[c]["out"][: len(oidx), :].astype(
                np.float32)
    return out.reshape(h_L.shape), res


def kernel(**inputs):
    out, _ = run(inputs, trace=False)
    return out
